# revision 2
# baseline (speedup 1.0000x reference)
"""Bass/Trainium2 kernel for nn_GCNNTemporal — row-sharded v3.

Reference (B=4 samples, O=8 objects, C=256, HID=128, H=W=64):
  states = relu(conv3x3(concat(feats, mask_o)))          per (sample, object)
  2x:  states_o = relu(conv3x3(concat(states_o, sum_{j!=o} states_j)))
  out_o = sigmoid(conv3x3(concat(feats, states_o)))

Sharding: 2 cores per sample split by IMAGE ROWS (32 rows each), all 8
objects per core. This dedups the per-sample shared convs (enc feats conv,
readout feats part, gcn conv(total)) across the pair with NO large
collectives: the only cross-core traffic is a 1-row boundary exchange per
conv step (AllReduce-add of the 8 objects' row-32 states; halo = sum -
own). SPMD symmetry: odd cores store their half vertically flipped (host
preps ky-flipped weights/inputs), so both run the identical program "send
local row 32, receive into local row 33".

Layout: 65-col rows with shared pad column; each conv buffer holds 34
local rows (l=0 zero guard / image boundary, l=1..32 own, l=33 exchanged
halo) between 66-elem zero guards. Conv3x3 SAME = 9 shifted matmuls into
PSUM with [8 rows x 64 cols] pad-skipping windows = 512 free elems = one
full PSUM bank, 4 chunks per conv. Algebra: gcn conv(concat(st,
total-st)) = conv(st, w1-w2) + conv(total, w2); total summed locally on
Vector (all 8 objects resident). Readout: 4 chunks x 90-slot psum chains
(8 obj x 9 taps block-diagonal M=8 + 2 feats ktiles x 9 taps broadcast),
chunks interleaved round-robin on 4 concurrent PE column strips; sigmoid
straight off PSUM. A dummy matmul chain at program start ramps the PE
clock during the input-DMA head.
"""
import sys
sys.path.insert(0, '/opt/trn_rl_repo')
import numpy as np

B, O, C, HID, H, W = 4, 8, 256, 128, 64, 64
STEPS = 2
N_CORES = 8

Wp = W + 1                  # 65
ROWS = 34                   # l=0 guard row, 1..32 own, 33 halo
NINT = ROWS * Wp            # 2210
GUARD = Wp + 1              # 66
EXT = GUARD + NINT + GUARD  # 2342
INT0 = GUARD
TAPS = [(ky, kx) for ky in range(3) for kx in range(3)]
NCH = 4                     # chunks per conv: 8 own rows each
CHUNKS = [Wp * (1 + 8 * i) for i in range(NCH)]   # nb of rows l=1+8i..8+8i
DN = 8 * W                  # 512 dense elems per chunk
NOUT = 32 * W               # 2048 out elems per object per core
GROUPS = [[0, 1], [2, 3], [4, 5], [6, 7]]

_PROG_CACHE = {}


def _build_program(repeat=1):
    import concourse.tile as tile
    from concourse import bacc, mybir

    AF = mybir.ActivationFunctionType
    F32 = mybir.dt.float32
    F16 = mybir.dt.float16

    nc = bacc.Bacc("TRN2", target_bir_lowering=False, debug=False,
                   num_devices=N_CORES)

    feats_ap = nc.dram_tensor("feats", [2, 128, EXT], F16, kind="ExternalInput").ap()
    mcols_ap = nc.dram_tensor("mcols", [8, 9, NOUT], F16, kind="ExternalInput").ap()
    encw_ap = nc.dram_tensor("encw", [128, 2 * 9 * 128], F16, kind="ExternalInput").ap()
    maskw_ap = nc.dram_tensor("maskw", [128, 128], F16, kind="ExternalInput").ap()
    gcnw12_ap = nc.dram_tensor("gcnw12", [128, 9 * 128], F16, kind="ExternalInput").ap()
    gcnw2_ap = nc.dram_tensor("gcnw2", [128, 9 * 128], F16, kind="ExternalInput").ap()
    row_ap = nc.dram_tensor("row", [128, 90 * 8], F16, kind="ExternalInput").ap()
    ident_ap = nc.dram_tensor("ident", [128, 128], F16, kind="ExternalInput").ap()
    encb_ap = nc.dram_tensor("encb", [128, 1], F32, kind="ExternalInput").ap()
    gcnb_ap = nc.dram_tensor("gcnb", [128, 1], F32, kind="ExternalInput").ap()
    rob_ap = nc.dram_tensor("rob", [128, 1], F32, kind="ExternalInput").ap()
    out_ap = nc.dram_tensor("out", [8, NOUT], F32, kind="ExternalOutput").ap()

    with tile.TileContext(nc) as tc:
        with tc.tile_pool(name="persist", bufs=1) as pp, \
             tc.tile_pool(name="psum", bufs=8, space="PSUM") as psp, \
             tc.tile_pool(name="dram", bufs=1, space="DRAM") as dp:

            # ---- persistent SBUF ----
            sts = pp.tile([128, 2, 8, EXT], F16, tag="sts")
            feats_sb = pp.tile([128, 2, EXT], F16, tag="feats")
            sh_sb = pp.tile([128, NOUT], F16, tag="sh")     # encF / gcn S, dense
            totals = [pp.tile([128, EXT], F16, tag=f"tot{k}", name=f"tot{k}")
                      for k in range(2)]
            mcols_sb = [pp.tile([128, NOUT], F16, tag=f"mc{g}", name=f"mc{g}")
                        for g in range(2)]
            arstage = pp.tile([128, 8, Wp], F16, tag="arstage")
            encw_sb = pp.tile([128, 2 * 9 * 128], F16, tag="encw")
            maskw_sb = pp.tile([128, 128], F16, tag="maskw")
            gcnw12_sb = pp.tile([128, 9 * 128], F16, tag="gw12")
            gcnw2_sb = pp.tile([128, 9 * 128], F16, tag="gw2")
            row_sb = pp.tile([128, 90 * 8], F16, tag="row")
            ident_sb = pp.tile([128, 128], F16, tag="ident")
            encb_sb = pp.tile([128, 1], F32, tag="encb")
            gcnb_sb = pp.tile([128, 1], F32, tag="gcnb")
            rob_sb = pp.tile([128, 1], F32, tag="rob")
            out_sb = pp.tile([128, NOUT], F32, tag="outsb")
            warm_sb = pp.tile([1, 1], F32, tag="warm")
            wz_sb = pp.tile([128, DN], F16, tag="wz")
            wsink_sb = pp.tile([128, DN], F16, tag="wsink")

            # ---- warm-up collective FIRST (CC stream init overlaps DMAs) ----
            ccw_in = dp.tile([1, 1], F32, tag="ccwin")
            ccw_out = dp.tile([1, 1], F32, tag="ccwout")
            nc.vector.memset(warm_sb[:], 0.0)
            nc.sync.dma_start(out=ccw_in[:], in_=warm_sb[:])
            nc.gpsimd.collective_compute(
                "AllReduce", mybir.AluOpType.add, replica_groups=GROUPS,
                ins=[ccw_in.opt()], outs=[ccw_out.opt()])

            # ---- PE clock warm-up: dummy chain during the DMA head ----
            nc.vector.memset(wz_sb[:], 0.0)
            wps = psp.tile([128, DN], F32, tag="cps", name="warmps")
            for wi in range(24):
                nc.tensor.matmul(wps[:], wz_sb[:, 0:128], wz_sb[:],
                                 start=(wi == 0), stop=(wi == 23))
            nc.scalar.activation(wsink_sb[:], wps[:], AF.Copy)

            # ---- input DMAs in consumption order (enc runs chunk 3 first,
            # so the feats tail ships before the head) ----
            nc.sync.dma_start(out=encw_sb[:], in_=encw_ap[:])
            for kt in range(2):
                nc.sync.dma_start(out=feats_sb[:, kt, 1560:2342],
                                  in_=feats_ap[kt, :, 1560:2342])
            nc.sync.dma_start(out=maskw_sb[:], in_=maskw_ap[:])
            nc.sync.dma_start(out=encb_sb[:], in_=encb_ap[:])
            for o in range(4):
                nc.sync.dma_start(out=mcols_sb[0][32 * o:32 * o + 9, :],
                                  in_=mcols_ap[o])
            for o in range(4):
                nc.sync.dma_start(out=mcols_sb[1][32 * o:32 * o + 9, :],
                                  in_=mcols_ap[4 + o])
            for kt in range(2):
                nc.sync.dma_start(out=feats_sb[:, kt, 0:1040],
                                  in_=feats_ap[kt, :, 0:1040])
            nc.sync.dma_start(out=ident_sb[:], in_=ident_ap[:])
            for kt in range(2):
                nc.sync.dma_start(out=feats_sb[:, kt, 1040:1560],
                                  in_=feats_ap[kt, :, 1040:1560])
            nc.sync.dma_start(out=gcnw12_sb[:], in_=gcnw12_ap[:])
            nc.sync.dma_start(out=gcnw2_sb[:], in_=gcnw2_ap[:])
            nc.sync.dma_start(out=gcnb_sb[:], in_=gcnb_ap[:])
            nc.sync.dma_start(out=row_sb[:], in_=row_ap[:])
            nc.sync.dma_start(out=rob_sb[:], in_=rob_ap[:])

            # ---- one-time zeroing: guards, l=0 row, pad cols, totals ----
            nc.vector.memset(sts[:, :, :, 0:INT0], 0.0)
            nc.vector.memset(sts[:, :, :, INT0 + NINT:EXT], 0.0)
            nc.vector.memset(sts[:, :, :, INT0:INT0 + Wp], 0.0)
            nc.vector.memset(sts[:, :, :, INT0 + W:INT0 + W + 33 * Wp + 1:Wp], 0.0)
            for tsb in totals:
                nc.vector.memset(tsb[:], 0.0)

            cc_ins = [dp.tile([128, 8, Wp], F16, tag=f"ccin{k}", name=f"ccin{k}")
                      for k in range(3)]
            cc_outs = [dp.tile([128, 8, Wp], F16, tag=f"ccout{k}", name=f"ccout{k}")
                       for k in range(3)]

            def st(ph, o):
                return sts[:, ph, o]

            def win(buf, nb, toff):
                """[128, 8, 64] pad-skipping conv window at out chunk nb, tap
                offset toff. buf is a full [128, EXT]-coords AP."""
                return buf[:, toff + nb:toff + nb + 8 * Wp].rearrange(
                    "p (r c) -> p r c", c=Wp)[:, :, 0:W]

            def skip(ap_ext, nb):
                """[128, 8, 64] interior write view of out chunk nb."""
                return ap_ext[:, INT0 + nb:INT0 + nb + 8 * Wp].rearrange(
                    "p (r c) -> p r c", c=Wp)[:, :, 0:W]

            def pv3(ps):
                return ps[:].rearrange("p (r c) -> p r c", c=W)

            def fire_ar(k, ph):
                nc.sync.dma_start(
                    out=cc_ins[k][:],
                    in_=sts[:, ph, :, INT0 + 32 * Wp:INT0 + 33 * Wp])
                nc.gpsimd.collective_compute(
                    "AllReduce", mybir.AluOpType.add, replica_groups=GROUPS,
                    ins=[cc_ins[k].opt()], outs=[cc_outs[k].opt()])

            def land_ar(k, ph, tot):
                """halo_o(l=33) = AR_o - own row32_o; optionally total row 33."""
                nc.sync.dma_start(out=arstage[:], in_=cc_outs[k][:])
                for o in range(8):
                    nc.vector.tensor_sub(
                        sts[:, ph, o, INT0 + 33 * Wp:INT0 + 34 * Wp],
                        arstage[:, o, :],
                        sts[:, ph, o, INT0 + 32 * Wp:INT0 + 33 * Wp])
                if tot is not None:
                    t33 = tot[:, INT0 + 33 * Wp:INT0 + 34 * Wp]
                    nc.vector.tensor_add(
                        t33, sts[:, ph, 0, INT0 + 33 * Wp:INT0 + 34 * Wp],
                        sts[:, ph, 1, INT0 + 33 * Wp:INT0 + 34 * Wp])
                    for o in range(2, 8):
                        nc.vector.tensor_add(
                            t33, t33, sts[:, ph, o, INT0 + 33 * Wp:INT0 + 34 * Wp])

            def accum_total(tot, ph, nb):
                tview = skip(tot, nb)
                nc.vector.tensor_add(tview, skip(st(ph, 0), nb),
                                     skip(st(ph, 1), nb))
                for o in range(2, 8):
                    nc.vector.tensor_add(tview, tview, skip(st(ph, o), nb))

            for _rep in range(repeat):
                # ================= ENC =================
                # boundary chunk (3) first so AR0 fires ~40us before its
                # consumer; per-chunk total via PE identity-matmul (enc is
                # vector-bound, PE has slack)
                with tc.tile_pool(name="encpool", bufs=1) as ep:
                    for ci in (3, 0, 1, 2):
                        nb = CHUNKS[ci]
                        dsl = slice(ci * DN, (ci + 1) * DN)
                        ps = psp.tile([128, DN], F32, tag="cps", name=f"esh{ci}")
                        for kt in range(2):
                            for t, (ky, kx) in enumerate(TAPS):
                                off = ky * Wp + kx
                                nc.tensor.matmul(
                                    ps[:].rearrange("p (r c) -> p r c", c=W),
                                    encw_sb[:, (kt * 9 + t) * 128:(kt * 9 + t + 1) * 128],
                                    win(feats_sb[:, kt], nb, off),
                                    start=(kt == 0 and t == 0), stop=(kt == 1 and t == 8))
                        nc.scalar.activation(sh_sb[:, dsl], ps[:], AF.Copy)
                        for g in range(2):
                            pss = [psp.tile([128, DN], F32, tag="cps",
                                            name=f"em{g}{j}c{ci}") for j in range(4)]
                            for j in range(4):
                                nc.tensor.matmul(
                                    pss[j][:], maskw_sb[32 * j:32 * j + 9, :],
                                    mcols_sb[g][32 * j:32 * j + 9, dsl],
                                    start=True, stop=True, tile_position=(32 * j, 0))
                            for j in range(4):
                                o = 4 * g + j
                                tmp = ep.tile([128, DN], F16, tag="etmp",
                                              bufs=4, name="etmp")
                                nc.vector.tensor_add(tmp[:], pss[j][:], sh_sb[:, dsl])
                                nc.scalar.activation(skip(st(0, o), nb),
                                                     pv3(tmp), AF.Relu, bias=encb_sb[:])
                        pt = psp.tile([128, DN], F32, tag="cps", name=f"et{ci}")
                        for o in range(8):
                            nc.tensor.matmul(pt[:].rearrange("p (r c) -> p r c", c=W),
                                             ident_sb[:], skip(st(0, o), nb),
                                             start=(o == 0), stop=(o == 7))
                        nc.vector.tensor_copy(skip(totals[0], nb),
                                              pt[:].rearrange("p (r c) -> p r c", c=W))
                        if ci == 3:
                            fire_ar(0, 0)

                # ================= GCN x2 =================
                for k in range(STEPS):
                    ph_in, ph_out = k % 2, (k + 1) % 2
                    tot_in = totals[k % 2]
                    tot_out = totals[1 - k % 2] if k == 0 else None
                    with tc.tile_pool(name=f"gcnpool{k}", bufs=1) as gp:
                        for ci in (0, 1, 3, 2):
                            nb = CHUNKS[ci]
                            if ci == NCH - 1:
                                land_ar(k, ph_in, tot_in)
                            dsl = slice(ci * DN, (ci + 1) * DN)
                            ps_sh = psp.tile([128, DN], F32, tag="cps",
                                             name=f"g{k}sh{ci}")
                            for t, (ky, kx) in enumerate(TAPS):
                                off = ky * Wp + kx
                                nc.tensor.matmul(
                                    ps_sh[:].rearrange("p (r c) -> p r c", c=W),
                                    gcnw2_sb[:, t * 128:(t + 1) * 128],
                                    win(tot_in, nb, off),
                                    start=(t == 0), stop=(t == 8))
                            nc.scalar.activation(sh_sb[:, dsl], ps_sh[:], AF.Copy)
                            for g in range(2):
                                objs = [4 * g + j for j in range(4)]
                                pss = [psp.tile([128, DN], F32, tag="cps",
                                                name=f"g{k}o{o}c{ci}") for o in objs]
                                for t, (ky, kx) in enumerate(TAPS):
                                    off = ky * Wp + kx
                                    for oi, o in enumerate(objs):
                                        nc.tensor.matmul(
                                            pss[oi][:].rearrange("p (r c) -> p r c", c=W),
                                            gcnw12_sb[:, t * 128:(t + 1) * 128],
                                            win(st(ph_in, o), nb, off),
                                            start=(t == 0), stop=(t == 8))
                                for oi, o in enumerate(objs):
                                    tmp = gp.tile([128, DN], F16, tag="gtmp",
                                                  bufs=4, name="gtmp")
                                    nc.vector.tensor_add(tmp[:], pss[oi][:],
                                                         sh_sb[:, dsl])
                                    nc.scalar.activation(skip(st(ph_out, o), nb),
                                                         pv3(tmp), AF.Relu,
                                                         bias=gcnb_sb[:])
                            if tot_out is not None:
                                accum_total(tot_out, ph_out, nb)
                            if ci == NCH - 1:
                                fire_ar(k + 1, ph_out)

                # ================= READOUT =================
                # 4 chunks interleaved on 4 PE column strips; 90-slot chains:
                # slot (src, t): src<8 = object (block-diag col), 8/9 = feats
                # ktile (broadcast). ky=2 taps last (halo-gated, chunk 3).
                slots = []
                for t in range(9):
                    for o in range(8):
                        slots.append((o, t))
                    for kt in range(2):
                        slots.append((8 + kt, t))
                land_ar(2, 0, None)
                pss = [psp.tile([128, DN], F32, tag="cps", name=f"ro{i}")
                       for i in range(NCH)]

                def ro_mm(i, si, src, t):
                    ky, kx = TAPS[t]
                    off = ky * Wp + kx
                    nb = CHUNKS[i]
                    if src < 8:
                        rhs = win(st(0, src), nb, off)
                        wsl = row_sb[:, (src * 9 + t) * 8:(src * 9 + t + 1) * 8]
                    else:
                        kt = src - 8
                        rhs = win(feats_sb[:, kt], nb, off)
                        wsl = row_sb[:, (72 + kt * 9 + t) * 8:(72 + kt * 9 + t + 1) * 8]
                    nc.tensor.matmul(
                        pss[i][32 * i:32 * i + 8, :].rearrange(
                            "p (r c) -> p r c", c=W),
                        wsl, rhs,
                        start=(si == 0), stop=(si == len(slots) - 1),
                        tile_position=(0, 32 * i))

                # interleave all 4 strips for the first 82 rounds, then emit
                # each chunk's tail + sigmoid + out-DMA staggered so the
                # finalize of chunks 0-2 hides under the remaining matmuls
                for si, (src, t) in enumerate(slots[:82]):
                    for i in range(NCH):
                        ro_mm(i, si, src, t)
                for i in range(NCH):
                    for si in range(82, 90):
                        src, t = slots[si]
                        ro_mm(i, si, src, t)
                    dsl = slice(i * DN, (i + 1) * DN)
                    nc.scalar.activation(out_sb[32 * i:32 * i + 8, dsl],
                                         pss[i][32 * i:32 * i + 8, :],
                                         AF.Sigmoid, bias=rob_sb[32 * i:32 * i + 8])
                    nc.sync.dma_start(out=out_ap[:, dsl],
                                      in_=out_sb[32 * i:32 * i + 8, dsl])

    nc.compile()
    return nc


def _host_prep(inputs):
    """Per-core input maps: row-shard + vertical flip (odd cores) + pad +
    dense im2col masks + lhsT weight layouts."""
    feats = np.asarray(inputs["batch_node_feats"], np.float32)
    masks = np.asarray(inputs["batch_previous_masks"], np.float32)
    enc_w = np.asarray(inputs["enc_w"], np.float32)
    enc_b = np.asarray(inputs["enc_b"], np.float32)
    gcn_w = np.asarray(inputs["gcn_w"], np.float32)
    gcn_b = np.asarray(inputs["gcn_b"], np.float32)
    ro_w = np.asarray(inputs["ro_w"], np.float32)
    ro_b = np.asarray(inputs["ro_b"], np.float32)

    def build_weights(flip):
        ew = enc_w[:, :, ::-1, :] if flip else enc_w
        gw = gcn_w[:, :, ::-1, :] if flip else gcn_w
        rw = ro_w[:, :, ::-1, :] if flip else ro_w
        encw = ew[:, :C].transpose(2, 3, 1, 0).reshape(9, 2, 128, HID) \
            .transpose(2, 1, 0, 3).reshape(128, 2 * 9 * HID)
        mvec = ew[:, C].transpose(1, 2, 0).reshape(9, HID)
        maskw = np.zeros((128, 128), np.float32)
        for j in range(4):
            maskw[32 * j:32 * j + 9] = mvec
        w1, w2 = gw[:, :HID], gw[:, HID:]
        gcnw12 = (w1 - w2).transpose(2, 3, 1, 0).reshape(9, 128, 128) \
            .transpose(1, 0, 2).reshape(128, 9 * 128)
        gcnw2 = w2.transpose(2, 3, 1, 0).reshape(9, 128, 128) \
            .transpose(1, 0, 2).reshape(128, 9 * 128)
        rs = rw[0, C:].transpose(1, 2, 0).reshape(9, HID)
        row = np.zeros((90, 128, 8), np.float32)
        for o in range(8):
            for t in range(9):
                row[o * 9 + t, :, o] = rs[t]
        for kt in range(2):
            sl = rw[0, kt * 128:(kt + 1) * 128].transpose(1, 2, 0).reshape(9, 128)
            for t in range(9):
                row[72 + kt * 9 + t] = sl[t][:, None]
        return (encw.astype(np.float16), maskw.astype(np.float16),
                gcnw12.astype(np.float16), gcnw2.astype(np.float16),
                row.transpose(1, 0, 2).reshape(128, 90 * 8).astype(np.float16))

    wsets = [build_weights(False), build_weights(True)]

    def rows34(img, flip):
        """[..., 64, 64] -> [..., 34, 64] local rows (l=0 zero guard)."""
        lead = img.shape[:-2]
        z = np.zeros(lead + (1, W), np.float32)
        if flip:
            body = img[..., 31:64, :][..., ::-1, :]
        else:
            body = img[..., 0:33, :]
        return np.concatenate([z, body], axis=-2)

    def flat65(img34):
        lead = img34.shape[:-2]
        fe = np.zeros(lead + (EXT,), np.float32)
        v = fe[..., INT0:INT0 + NINT].reshape(lead + (ROWS, Wp))
        v[..., :, 0:W] = img34
        return fe

    encb = enc_b.reshape(128, 1).astype(np.float32)
    gcnb = gcn_b.reshape(128, 1).astype(np.float32)
    rob = np.full((128, 1), ro_b[0], np.float32)

    in_maps = []
    for c in range(N_CORES):
        s, flip = c // 2, bool(c % 2)
        encw, maskw, gcnw12, gcnw2, row = wsets[c % 2]
        fe = flat65(rows34(feats[s], flip))             # [256, EXT]
        m34 = rows34(masks[s], flip)                    # [8, 34, 64]
        # dense im2col: mc[o, t, r*64+x] = mask[local row r+ky, col x-1+kx]
        mp = np.pad(m34, ((0, 0), (0, 0), (1, 1)))      # pad x by 1
        mc = np.zeros((8, 9, NOUT), np.float32)
        for t, (ky, kx) in enumerate(TAPS):
            mc[:, t] = mp[:, ky:ky + 32, kx:kx + W].reshape(8, NOUT)
        in_maps.append({
            "feats": fe.reshape(2, 128, EXT).astype(np.float16),
            "mcols": mc.astype(np.float16),
            "encw": encw, "maskw": maskw,
            "gcnw12": gcnw12, "gcnw2": gcnw2, "row": row,
            "ident": np.eye(128, dtype=np.float16),
            "encb": encb, "gcnb": gcnb, "rob": rob,
        })
    return in_maps


def _run(inputs, repeat=1):
    from concourse.bass_utils import run_bass_kernel_spmd
    if repeat not in _PROG_CACHE:
        _PROG_CACHE[repeat] = _build_program(repeat)
    nc = _PROG_CACHE[repeat]
    in_maps = _host_prep(inputs)
    r = run_bass_kernel_spmd(nc, in_maps, list(range(N_CORES)))
    out = np.zeros((B, O, H, W), np.float32)
    for c in range(N_CORES):
        s, flip = c // 2, c % 2
        res = r.results[c]["out"].reshape(8, 32, 64)
        if flip:
            out[s, :, 32:64] = res[:, ::-1, :]
        else:
            out[s, :, 0:32] = res
    return out


def kernel(**inputs) -> np.ndarray:
    return _run(inputs, repeat=1)


# revision 3
# speedup vs baseline: 1.0250x; 1.0250x over previous
"""Bass/Trainium2 kernel for nn_GCNNTemporal — row-sharded v3.

Reference (B=4 samples, O=8 objects, C=256, HID=128, H=W=64):
  states = relu(conv3x3(concat(feats, mask_o)))          per (sample, object)
  2x:  states_o = relu(conv3x3(concat(states_o, sum_{j!=o} states_j)))
  out_o = sigmoid(conv3x3(concat(feats, states_o)))

Sharding: 2 cores per sample split by IMAGE ROWS (32 rows each), all 8
objects per core. This dedups the per-sample shared convs (enc feats conv,
readout feats part, gcn conv(total)) across the pair with NO large
collectives: the only cross-core traffic is a 1-row boundary exchange per
conv step (AllReduce-add of the 8 objects' row-32 states; halo = sum -
own). SPMD symmetry: odd cores store their half vertically flipped (host
preps ky-flipped weights/inputs), so both run the identical program "send
local row 32, receive into local row 33".

Layout: 65-col rows with shared pad column; each conv buffer holds 34
local rows (l=0 zero guard / image boundary, l=1..32 own, l=33 exchanged
halo) between 66-elem zero guards. Conv3x3 SAME = 9 shifted matmuls into
PSUM with [8 rows x 64 cols] pad-skipping windows = 512 free elems = one
full PSUM bank, 4 chunks per conv. Algebra: gcn conv(concat(st,
total-st)) = conv(st, w1-w2) + conv(total, w2); total summed locally on
Vector (all 8 objects resident). Readout: 4 chunks x 90-slot psum chains
(8 obj x 9 taps block-diagonal M=8 + 2 feats ktiles x 9 taps broadcast),
chunks interleaved round-robin on 4 concurrent PE column strips; sigmoid
straight off PSUM. A dummy matmul chain at program start ramps the PE
clock during the input-DMA head.
"""
import sys
sys.path.insert(0, '/opt/trn_rl_repo')
import numpy as np

B, O, C, HID, H, W = 4, 8, 256, 128, 64, 64
STEPS = 2
N_CORES = 8

Wp = W + 1                  # 65
ROWS = 34                   # l=0 guard row, 1..32 own, 33 halo
NINT = ROWS * Wp            # 2210
GUARD = Wp + 1              # 66
EXT = GUARD + NINT + GUARD  # 2342
INT0 = GUARD
TAPS = [(ky, kx) for ky in range(3) for kx in range(3)]
NCH = 4                     # chunks per conv: 8 own rows each
CHUNKS = [Wp * (1 + 8 * i) for i in range(NCH)]   # nb of rows l=1+8i..8+8i
DN = 8 * W                  # 512 dense elems per chunk
NOUT = 32 * W               # 2048 out elems per object per core
GROUPS = [[0, 1], [2, 3], [4, 5], [6, 7]]

_PROG_CACHE = {}


def _build_program(repeat=1):
    import concourse.tile as tile
    from concourse import bacc, mybir

    AF = mybir.ActivationFunctionType
    F32 = mybir.dt.float32
    F16 = mybir.dt.float16

    nc = bacc.Bacc("TRN2", target_bir_lowering=False, debug=False,
                   num_devices=N_CORES)

    feats_ap = nc.dram_tensor("feats", [2, 128, EXT], F16, kind="ExternalInput").ap()
    mcols_ap = nc.dram_tensor("mcols", [8, 9, NOUT], F16, kind="ExternalInput").ap()
    encw_ap = nc.dram_tensor("encw", [128, 2 * 9 * 128], F16, kind="ExternalInput").ap()
    maskw_ap = nc.dram_tensor("maskw", [128, 128], F16, kind="ExternalInput").ap()
    gcnw12_ap = nc.dram_tensor("gcnw12", [128, 9 * 128], F16, kind="ExternalInput").ap()
    gcnw2_ap = nc.dram_tensor("gcnw2", [128, 9 * 128], F16, kind="ExternalInput").ap()
    row_ap = nc.dram_tensor("row", [128, 90 * 8], F16, kind="ExternalInput").ap()
    ident_ap = nc.dram_tensor("ident", [128, 128], F16, kind="ExternalInput").ap()
    encb_ap = nc.dram_tensor("encb", [128, 1], F32, kind="ExternalInput").ap()
    gcnb_ap = nc.dram_tensor("gcnb", [128, 1], F32, kind="ExternalInput").ap()
    rob_ap = nc.dram_tensor("rob", [128, 1], F32, kind="ExternalInput").ap()
    out_ap = nc.dram_tensor("out", [8, NOUT], F32, kind="ExternalOutput").ap()

    with tile.TileContext(nc) as tc:
        with tc.tile_pool(name="persist", bufs=1) as pp, \
             tc.tile_pool(name="psum", bufs=8, space="PSUM") as psp, \
             tc.tile_pool(name="dram", bufs=1, space="DRAM") as dp:

            # ---- persistent SBUF ----
            sts = pp.tile([128, 2, 8, EXT], F16, tag="sts")
            feats_sb = pp.tile([128, 2, EXT], F16, tag="feats")
            sh_sb = pp.tile([128, NOUT], F16, tag="sh")     # encF / gcn S, dense
            totals = [pp.tile([128, EXT], F16, tag=f"tot{k}", name=f"tot{k}")
                      for k in range(2)]
            mcols_sb = [pp.tile([128, NOUT], F16, tag=f"mc{g}", name=f"mc{g}")
                        for g in range(2)]
            arstage = pp.tile([128, 8, Wp], F16, tag="arstage")
            encw_sb = pp.tile([128, 2 * 9 * 128], F16, tag="encw")
            maskw_sb = pp.tile([128, 128], F16, tag="maskw")
            gcnw12_sb = pp.tile([128, 9 * 128], F16, tag="gw12")
            gcnw2_sb = pp.tile([128, 9 * 128], F16, tag="gw2")
            row_sb = pp.tile([128, 90 * 8], F16, tag="row")
            ident_sb = pp.tile([128, 128], F16, tag="ident")
            encb_sb = pp.tile([128, 1], F32, tag="encb")
            gcnb_sb = pp.tile([128, 1], F32, tag="gcnb")
            rob_sb = pp.tile([128, 1], F32, tag="rob")
            out_sb = pp.tile([128, NOUT], F32, tag="outsb")
            warm_sb = pp.tile([1, 1], F32, tag="warm")
            wz_sb = pp.tile([128, DN], F16, tag="wz")
            wsink_sb = pp.tile([128, DN], F16, tag="wsink")

            # ---- warm-up collective FIRST (CC stream init overlaps DMAs) ----
            ccw_in = dp.tile([1, 1], F32, tag="ccwin")
            ccw_out = dp.tile([1, 1], F32, tag="ccwout")
            nc.vector.memset(warm_sb[:], 0.0)
            nc.sync.dma_start(out=ccw_in[:], in_=warm_sb[:])
            nc.gpsimd.collective_compute(
                "AllReduce", mybir.AluOpType.add, replica_groups=GROUPS,
                ins=[ccw_in.opt()], outs=[ccw_out.opt()])

            # ---- PE clock warm-up: dummy chain during the DMA head ----
            nc.vector.memset(wz_sb[:], 0.0)
            wps = psp.tile([128, DN], F32, tag="cps", name="warmps")
            for wi in range(24):
                nc.tensor.matmul(wps[:], wz_sb[:, 0:128], wz_sb[:],
                                 start=(wi == 0), stop=(wi == 23))
            nc.scalar.activation(wsink_sb[:], wps[:], AF.Copy)

            # ---- input DMAs in consumption order (enc runs chunk 3 first,
            # so the feats tail ships before the head) ----
            nc.sync.dma_start(out=encw_sb[:], in_=encw_ap[:])
            for kt in range(2):
                nc.sync.dma_start(out=feats_sb[:, kt, 1560:2342],
                                  in_=feats_ap[kt, :, 1560:2342])
            nc.sync.dma_start(out=maskw_sb[:], in_=maskw_ap[:])
            nc.sync.dma_start(out=encb_sb[:], in_=encb_ap[:])
            for o in range(4):
                nc.sync.dma_start(out=mcols_sb[0][32 * o:32 * o + 9, :],
                                  in_=mcols_ap[o])
            for o in range(4):
                nc.sync.dma_start(out=mcols_sb[1][32 * o:32 * o + 9, :],
                                  in_=mcols_ap[4 + o])
            for kt in range(2):
                nc.sync.dma_start(out=feats_sb[:, kt, 0:1040],
                                  in_=feats_ap[kt, :, 0:1040])
            nc.sync.dma_start(out=ident_sb[:], in_=ident_ap[:])
            for kt in range(2):
                nc.sync.dma_start(out=feats_sb[:, kt, 1040:1560],
                                  in_=feats_ap[kt, :, 1040:1560])
            nc.sync.dma_start(out=gcnw12_sb[:], in_=gcnw12_ap[:])
            nc.sync.dma_start(out=gcnw2_sb[:], in_=gcnw2_ap[:])
            nc.sync.dma_start(out=gcnb_sb[:], in_=gcnb_ap[:])
            nc.sync.dma_start(out=row_sb[:], in_=row_ap[:])
            nc.sync.dma_start(out=rob_sb[:], in_=rob_ap[:])

            # ---- one-time zeroing: guards, l=0 row, pad cols, totals ----
            nc.vector.memset(sts[:, :, :, 0:INT0], 0.0)
            nc.vector.memset(sts[:, :, :, INT0 + NINT:EXT], 0.0)
            nc.vector.memset(sts[:, :, :, INT0:INT0 + Wp], 0.0)
            nc.vector.memset(sts[:, :, :, INT0 + W:INT0 + W + 33 * Wp + 1:Wp], 0.0)
            for tsb in totals:
                nc.vector.memset(tsb[:], 0.0)

            cc_ins = [dp.tile([128, 8, Wp], F16, tag=f"ccin{k}", name=f"ccin{k}")
                      for k in range(3)]
            cc_outs = [dp.tile([128, 8, Wp], F16, tag=f"ccout{k}", name=f"ccout{k}")
                       for k in range(3)]

            def st(ph, o):
                return sts[:, ph, o]

            def win(buf, nb, toff):
                """[128, 8, 64] pad-skipping conv window at out chunk nb, tap
                offset toff. buf is a full [128, EXT]-coords AP."""
                return buf[:, toff + nb:toff + nb + 8 * Wp].rearrange(
                    "p (r c) -> p r c", c=Wp)[:, :, 0:W]

            def skip(ap_ext, nb):
                """[128, 8, 64] interior write view of out chunk nb."""
                return ap_ext[:, INT0 + nb:INT0 + nb + 8 * Wp].rearrange(
                    "p (r c) -> p r c", c=Wp)[:, :, 0:W]

            def pv3(ps):
                return ps[:].rearrange("p (r c) -> p r c", c=W)

            def fire_ar(k, ph):
                nc.sync.dma_start(
                    out=cc_ins[k][:],
                    in_=sts[:, ph, :, INT0 + 32 * Wp:INT0 + 33 * Wp])
                nc.gpsimd.collective_compute(
                    "AllReduce", mybir.AluOpType.add, replica_groups=GROUPS,
                    ins=[cc_ins[k].opt()], outs=[cc_outs[k].opt()])

            def land_ar(k, ph, tot):
                """halo_o(l=33) = AR_o - own row32_o; optionally total row 33."""
                nc.sync.dma_start(out=arstage[:], in_=cc_outs[k][:])
                for o in range(8):
                    nc.vector.tensor_sub(
                        sts[:, ph, o, INT0 + 33 * Wp:INT0 + 34 * Wp],
                        arstage[:, o, :],
                        sts[:, ph, o, INT0 + 32 * Wp:INT0 + 33 * Wp])
                if tot is not None:
                    t33 = tot[:, INT0 + 33 * Wp:INT0 + 34 * Wp]
                    nc.vector.tensor_add(
                        t33, sts[:, ph, 0, INT0 + 33 * Wp:INT0 + 34 * Wp],
                        sts[:, ph, 1, INT0 + 33 * Wp:INT0 + 34 * Wp])
                    for o in range(2, 8):
                        nc.vector.tensor_add(
                            t33, t33, sts[:, ph, o, INT0 + 33 * Wp:INT0 + 34 * Wp])

            def accum_total(tot, ph, nb):
                tview = skip(tot, nb)
                nc.vector.tensor_add(tview, skip(st(ph, 0), nb),
                                     skip(st(ph, 1), nb))
                for o in range(2, 8):
                    nc.vector.tensor_add(tview, tview, skip(st(ph, o), nb))

            for _rep in range(repeat):
                # ================= ENC =================
                # boundary chunk (3) first so AR0 fires ~40us before its
                # consumer; per-chunk total via PE identity-matmul (enc is
                # vector-bound, PE has slack)
                # per object: psum = mask conv + I128 x encF (2-matmul chain),
                # relu+bias straight off PSUM on scalar — no vector add; the
                # freed vector engine does the 8-object totals.
                for ci in (3, 0, 1, 2):
                    nb = CHUNKS[ci]
                    dsl = slice(ci * DN, (ci + 1) * DN)
                    ps = psp.tile([128, DN], F32, tag="cps", name=f"esh{ci}")
                    for kt in range(2):
                        for t, (ky, kx) in enumerate(TAPS):
                            off = ky * Wp + kx
                            nc.tensor.matmul(
                                ps[:].rearrange("p (r c) -> p r c", c=W),
                                encw_sb[:, (kt * 9 + t) * 128:(kt * 9 + t + 1) * 128],
                                win(feats_sb[:, kt], nb, off),
                                start=(kt == 0 and t == 0), stop=(kt == 1 and t == 8))
                    nc.scalar.activation(sh_sb[:, dsl], ps[:], AF.Copy)
                    for g in range(2):
                        pss = [psp.tile([128, DN], F32, tag="cps",
                                        name=f"em{g}{j}c{ci}") for j in range(4)]
                        for j in range(4):
                            nc.tensor.matmul(
                                pss[j][:], maskw_sb[32 * j:32 * j + 9, :],
                                mcols_sb[g][32 * j:32 * j + 9, dsl],
                                start=True, stop=False, tile_position=(32 * j, 0))
                        for j in range(4):
                            nc.tensor.matmul(
                                pss[j][:], ident_sb[:], sh_sb[:, dsl],
                                start=False, stop=True, tile_position=(0, 0))
                        for j in range(4):
                            o = 4 * g + j
                            nc.scalar.activation(skip(st(0, o), nb),
                                                 pss[j][:].rearrange(
                                                     "p (r c) -> p r c", c=W),
                                                 AF.Relu, bias=encb_sb[:])
                    accum_total(totals[0], 0, nb)
                    if ci == 3:
                        fire_ar(0, 0)

                # ================= GCN x2 =================
                for k in range(STEPS):
                    ph_in, ph_out = k % 2, (k + 1) % 2
                    tot_in = totals[k % 2]
                    tot_out = totals[1 - k % 2] if k == 0 else None
                    with tc.tile_pool(name=f"gcnpool{k}", bufs=1) as gp:
                        for ci in (0, 1, 3, 2):
                            nb = CHUNKS[ci]
                            if ci == NCH - 1:
                                land_ar(k, ph_in, tot_in)
                            dsl = slice(ci * DN, (ci + 1) * DN)
                            ps_sh = psp.tile([128, DN], F32, tag="cps",
                                             name=f"g{k}sh{ci}")
                            for t, (ky, kx) in enumerate(TAPS):
                                off = ky * Wp + kx
                                nc.tensor.matmul(
                                    ps_sh[:].rearrange("p (r c) -> p r c", c=W),
                                    gcnw2_sb[:, t * 128:(t + 1) * 128],
                                    win(tot_in, nb, off),
                                    start=(t == 0), stop=(t == 8))
                            nc.scalar.activation(sh_sb[:, dsl], ps_sh[:], AF.Copy)
                            for g in range(2):
                                objs = [4 * g + j for j in range(4)]
                                pss = [psp.tile([128, DN], F32, tag="cps",
                                                name=f"g{k}o{o}c{ci}") for o in objs]
                                for t, (ky, kx) in enumerate(TAPS):
                                    off = ky * Wp + kx
                                    for oi, o in enumerate(objs):
                                        nc.tensor.matmul(
                                            pss[oi][:].rearrange("p (r c) -> p r c", c=W),
                                            gcnw12_sb[:, t * 128:(t + 1) * 128],
                                            win(st(ph_in, o), nb, off),
                                            start=(t == 0), stop=(t == 8))
                                for oi, o in enumerate(objs):
                                    tmp = gp.tile([128, DN], F16, tag="gtmp",
                                                  bufs=4, name="gtmp")
                                    nc.vector.tensor_add(tmp[:], pss[oi][:],
                                                         sh_sb[:, dsl])
                                    nc.scalar.activation(skip(st(ph_out, o), nb),
                                                         pv3(tmp), AF.Relu,
                                                         bias=gcnb_sb[:])
                            if tot_out is not None:
                                accum_total(tot_out, ph_out, nb)
                            if ci == NCH - 1:
                                fire_ar(k + 1, ph_out)

                # ================= READOUT =================
                # 4 chunks interleaved on 4 PE column strips; 90-slot chains:
                # slot (src, t): src<8 = object (block-diag col), 8/9 = feats
                # ktile (broadcast). ky=2 taps last (halo-gated, chunk 3).
                slots = []
                for t in range(9):
                    for o in range(8):
                        slots.append((o, t))
                    for kt in range(2):
                        slots.append((8 + kt, t))
                land_ar(2, 0, None)
                pss = [psp.tile([128, DN], F32, tag="cps", name=f"ro{i}")
                       for i in range(NCH)]

                def ro_mm(i, si, src, t):
                    ky, kx = TAPS[t]
                    off = ky * Wp + kx
                    nb = CHUNKS[i]
                    if src < 8:
                        rhs = win(st(0, src), nb, off)
                        wsl = row_sb[:, (src * 9 + t) * 8:(src * 9 + t + 1) * 8]
                    else:
                        kt = src - 8
                        rhs = win(feats_sb[:, kt], nb, off)
                        wsl = row_sb[:, (72 + kt * 9 + t) * 8:(72 + kt * 9 + t + 1) * 8]
                    nc.tensor.matmul(
                        pss[i][32 * i:32 * i + 8, :].rearrange(
                            "p (r c) -> p r c", c=W),
                        wsl, rhs,
                        start=(si == 0), stop=(si == len(slots) - 1),
                        tile_position=(0, 32 * i))

                # interleave all 4 strips for the first 82 rounds, then emit
                # each chunk's tail + sigmoid + out-DMA staggered so the
                # finalize of chunks 0-2 hides under the remaining matmuls
                for si, (src, t) in enumerate(slots[:82]):
                    for i in range(NCH):
                        ro_mm(i, si, src, t)
                for i in (3, 0, 1, 2):
                    for si in range(82, 90):
                        src, t = slots[si]
                        ro_mm(i, si, src, t)
                    dsl = slice(i * DN, (i + 1) * DN)
                    nc.scalar.activation(out_sb[32 * i:32 * i + 8, dsl],
                                         pss[i][32 * i:32 * i + 8, :],
                                         AF.Sigmoid, bias=rob_sb[32 * i:32 * i + 8])
                    nc.sync.dma_start(out=out_ap[:, dsl],
                                      in_=out_sb[32 * i:32 * i + 8, dsl])

    nc.compile()
    return nc


def _host_prep(inputs):
    """Per-core input maps: row-shard + vertical flip (odd cores) + pad +
    dense im2col masks + lhsT weight layouts."""
    feats = np.asarray(inputs["batch_node_feats"], np.float32)
    masks = np.asarray(inputs["batch_previous_masks"], np.float32)
    enc_w = np.asarray(inputs["enc_w"], np.float32)
    enc_b = np.asarray(inputs["enc_b"], np.float32)
    gcn_w = np.asarray(inputs["gcn_w"], np.float32)
    gcn_b = np.asarray(inputs["gcn_b"], np.float32)
    ro_w = np.asarray(inputs["ro_w"], np.float32)
    ro_b = np.asarray(inputs["ro_b"], np.float32)

    def build_weights(flip):
        ew = enc_w[:, :, ::-1, :] if flip else enc_w
        gw = gcn_w[:, :, ::-1, :] if flip else gcn_w
        rw = ro_w[:, :, ::-1, :] if flip else ro_w
        encw = ew[:, :C].transpose(2, 3, 1, 0).reshape(9, 2, 128, HID) \
            .transpose(2, 1, 0, 3).reshape(128, 2 * 9 * HID)
        mvec = ew[:, C].transpose(1, 2, 0).reshape(9, HID)
        maskw = np.zeros((128, 128), np.float32)
        for j in range(4):
            maskw[32 * j:32 * j + 9] = mvec
        w1, w2 = gw[:, :HID], gw[:, HID:]
        gcnw12 = (w1 - w2).transpose(2, 3, 1, 0).reshape(9, 128, 128) \
            .transpose(1, 0, 2).reshape(128, 9 * 128)
        gcnw2 = w2.transpose(2, 3, 1, 0).reshape(9, 128, 128) \
            .transpose(1, 0, 2).reshape(128, 9 * 128)
        rs = rw[0, C:].transpose(1, 2, 0).reshape(9, HID)
        row = np.zeros((90, 128, 8), np.float32)
        for o in range(8):
            for t in range(9):
                row[o * 9 + t, :, o] = rs[t]
        for kt in range(2):
            sl = rw[0, kt * 128:(kt + 1) * 128].transpose(1, 2, 0).reshape(9, 128)
            for t in range(9):
                row[72 + kt * 9 + t] = sl[t][:, None]
        return (encw.astype(np.float16), maskw.astype(np.float16),
                gcnw12.astype(np.float16), gcnw2.astype(np.float16),
                row.transpose(1, 0, 2).reshape(128, 90 * 8).astype(np.float16))

    wsets = [build_weights(False), build_weights(True)]

    def rows34(img, flip):
        """[..., 64, 64] -> [..., 34, 64] local rows (l=0 zero guard)."""
        lead = img.shape[:-2]
        z = np.zeros(lead + (1, W), np.float32)
        if flip:
            body = img[..., 31:64, :][..., ::-1, :]
        else:
            body = img[..., 0:33, :]
        return np.concatenate([z, body], axis=-2)

    def flat65(img34):
        lead = img34.shape[:-2]
        fe = np.zeros(lead + (EXT,), np.float32)
        v = fe[..., INT0:INT0 + NINT].reshape(lead + (ROWS, Wp))
        v[..., :, 0:W] = img34
        return fe

    encb = enc_b.reshape(128, 1).astype(np.float32)
    gcnb = gcn_b.reshape(128, 1).astype(np.float32)
    rob = np.full((128, 1), ro_b[0], np.float32)

    in_maps = []
    for c in range(N_CORES):
        s, flip = c // 2, bool(c % 2)
        encw, maskw, gcnw12, gcnw2, row = wsets[c % 2]
        fe = flat65(rows34(feats[s], flip))             # [256, EXT]
        m34 = rows34(masks[s], flip)                    # [8, 34, 64]
        # dense im2col: mc[o, t, r*64+x] = mask[local row r+ky, col x-1+kx]
        mp = np.pad(m34, ((0, 0), (0, 0), (1, 1)))      # pad x by 1
        mc = np.zeros((8, 9, NOUT), np.float32)
        for t, (ky, kx) in enumerate(TAPS):
            mc[:, t] = mp[:, ky:ky + 32, kx:kx + W].reshape(8, NOUT)
        in_maps.append({
            "feats": fe.reshape(2, 128, EXT).astype(np.float16),
            "mcols": mc.astype(np.float16),
            "encw": encw, "maskw": maskw,
            "gcnw12": gcnw12, "gcnw2": gcnw2, "row": row,
            "ident": np.eye(128, dtype=np.float16),
            "encb": encb, "gcnb": gcnb, "rob": rob,
        })
    return in_maps


def _run(inputs, repeat=1):
    from concourse.bass_utils import run_bass_kernel_spmd
    if repeat not in _PROG_CACHE:
        _PROG_CACHE[repeat] = _build_program(repeat)
    nc = _PROG_CACHE[repeat]
    in_maps = _host_prep(inputs)
    r = run_bass_kernel_spmd(nc, in_maps, list(range(N_CORES)))
    out = np.zeros((B, O, H, W), np.float32)
    for c in range(N_CORES):
        s, flip = c // 2, c % 2
        res = r.results[c]["out"].reshape(8, 32, 64)
        if flip:
            out[s, :, 32:64] = res[:, ::-1, :]
        else:
            out[s, :, 0:32] = res
    return out


def kernel(**inputs) -> np.ndarray:
    return _run(inputs, repeat=1)
